# revision 8
# baseline (speedup 1.0000x reference)
"""Trainium2 Bass kernel for the PINN-style loss problem (v2, fp16 tower).

Math: a 6-layer tanh MLP u(x,t) (2->50x5->1) is evaluated with forward-mode
jets (u, u_x, u_t, u_xxx) at N=10000 points. The per-param loss
  loss_p = mean_n (u_t + a_p*u*u_x + b_p*u_xxx + c_p*u_x)^2
collapses to a quadratic form in the 4x4 Gram matrix of
v_n = [u*u_x, u_xxx, u_x, u_t]:  loss_p = sum_ij p_i p_j G_ij / N with
p = [a_p, b_p, c_p, 1].

Sharding: x is split into 8 slices of 1250 points (one per NeuronCore);
each core builds its partial Gram, an AllReduce sums them, then each core
contracts the global Gram against host-precomputed para features
Q16[16,625] (rows p_i*p_j) with a single tiny matmul.

Device layout: points are packed 2-per-partition-block (block-diagonal
weights, K=100), free dim 640 per block (block0: 640 real points,
block1: 610 real + 30 zero-padded, masked out before the Gram matmul).
The jet pipeline runs in fp16 (DVE 2x / PE 16-bit rates); PSUM stays f32.
"""

import os
import sys
import numpy as np

for _p in ("/opt/trn_rl_repo",):
    if os.path.isdir(_p) and _p not in sys.path:
        sys.path.append(_p)

import concourse.bass as bass
import concourse.bacc as bacc
import concourse.mybir as mybir
import concourse.tile as tile
from concourse import bass_utils

F32 = mybir.dt.float32
F16 = mybir.dt.float16
AF = mybir.ActivationFunctionType
ALU = mybir.AluOpType

NCORES = 8
NPTS = 10000
NPC = NPTS // NCORES       # 1250 points per core
PPC = 5000 // NCORES       # 625 para rows per core
FD = 640                   # free dim per block (block0 full, block1 padded)
B1 = NPC - FD              # 610 real points in block1
HB = 100                   # 2 blocks x 50 hidden units
CHUNKS = ((0, 512), (512, 128))      # matmul free-dim chunks (psum bank limit)
QCH = ((0, 512), (512, PPC - 512))   # loss free-dim chunks

WARM_CC = True             # early dummy collective to warm the CC path


def _mm_chunks(nc, out_tile, lhsT, rhs_tile, chunks=CHUNKS):
    for off, w in chunks:
        nc.tensor.matmul(out_tile[:, off:off + w], lhsT, rhs_tile[:, off:off + w])


def build_program(stage="full"):
    nc = bacc.Bacc("TRN2", target_bir_lowering=False, debug=False)

    cstA_d = nc.dram_tensor("cstA", [HB, 402], F16, kind="ExternalInput")
    cstB_d = nc.dram_tensor("cstB", [16, 1365], F16, kind="ExternalInput")
    cstF_d = nc.dram_tensor("cstF", [128, 12], F32, kind="ExternalInput")
    if stage == "tower":
        loss_d = nc.dram_tensor("dbg", [HB, FD], F32, kind="ExternalOutput")
    elif stage == "l6":
        loss_d = nc.dram_tensor("dbg", [4, 4], F32, kind="ExternalOutput")
    else:
        loss_d = nc.dram_tensor("loss", [1, PPC], F32, kind="ExternalOutput")

    with tile.TileContext(nc) as tc:
        _body(tc, nc, cstA_d, cstB_d, cstF_d, loss_d, stage=stage)
    nc.compile()
    return nc


def _body(tc, nc, cstA_d, cstB_d, cstF_d, loss_d, stage="full"):
    import contextlib

    ctx = contextlib.ExitStack()
    with ctx:
        cpool = ctx.enter_context(tc.tile_pool(name="const", bufs=1))
        spool = ctx.enter_context(tc.tile_pool(name="streams", bufs=2))
        tpool = ctx.enter_context(tc.tile_pool(name="trans", bufs=2))
        dpool = ctx.enter_context(tc.tile_pool(name="dram", bufs=1, space="DRAM"))

        # ---- load constants (3 batched DMAs) ----
        A = cpool.tile([HB, 402], F16, tag="cstA")
        B = cpool.tile([16, 1365], F16, tag="cstB")
        F = cpool.tile([128, 12], F32, tag="cstF")
        nc.sync.dma_start(B[:], cstB_d[:])
        nc.sync.dma_start(A[:], cstA_d[:])
        nc.sync.dma_start(F[:], cstF_d[:])

        h0 = B[0:4, 0:640]
        w1t = B[0:4, 640:740]
        q16 = B[:, 740:1365]
        w6p = A[:, 400:402]

        def wl(layer):  # weight block for layer 2..5
            return A[:, 100 * (layer - 2):100 * (layer - 1)]

        cx = F[0:HB, 0:1]
        ct = F[0:HB, 1:2]
        cx2 = F[0:HB, 2:3]
        cx3 = F[0:HB, 3:4]

        def bb(layer):  # bias vector for layer 1..5
            return F[0:HB, 3 + layer:4 + layer]

        b6 = F[:, 10:11]
        msk = F[:, 11:12]

        wone = cpool.tile([1, 1], F32, tag="wone")
        nc.vector.memset(wone[:], 1.0)

        if WARM_CC:
            win = dpool.tile([1, 1], F32, tag="win")
            wout = dpool.tile([1, 1], F32, tag="wout")
            nc.gpsimd.dma_start(win[:], wone[:])
            nc.gpsimd.collective_compute(
                "AllReduce", ALU.add,
                replica_groups=[list(range(NCORES))],
                ins=[win.opt()], outs=[wout.opt()],
            )

        v = nc.vector
        s = nc.scalar
        g = nc.gpsimd

        a5 = ax5 = at5 = axxx5 = None

        with tc.tile_pool(name="ztw", bufs=3, space="PSUM") as zpool:
            # ---------- layer 1 ----------
            # zx/zt are constant per hidden unit: cx/ct. Jets come from
            # tensor_scalar ops with the per-partition weight columns.
            z = zpool.tile([HB, FD], F32, tag="ztw")
            _mm_chunks(nc, z, w1t, h0)
            a = spool.tile([HB, FD], F16, tag="a")
            s.activation(a[:], z[:], AF.Tanh, bias=bb(1))
            asq = tpool.tile([HB, FD], F16, tag="asq")
            s.activation(asq[:], a[:], AF.Square)
            f1 = tpool.tile([HB, FD], F16, tag="f1")
            v.tensor_scalar(f1[:], asq[:], -1.0, 1.0, ALU.mult, ALU.add)
            h6 = tpool.tile([HB, FD], F16, tag="h6")
            v.tensor_scalar(h6[:], asq[:], 6.0, -2.0, ALU.mult, ALU.add)
            ax = spool.tile([HB, FD], F16, tag="ax")
            v.tensor_scalar(ax[:], f1[:], cx, None, ALU.mult)
            at = spool.tile([HB, FD], F16, tag="at")
            v.tensor_scalar(at[:], f1[:], ct, None, ALU.mult)
            af1 = tpool.tile([HB, FD], F16, tag="t2")
            v.tensor_tensor(af1[:], a[:], f1[:], ALU.mult)
            axx = spool.tile([HB, FD], F16, tag="axx")
            v.tensor_scalar(axx[:], af1[:], cx2, -2.0, ALU.mult, ALU.mult)
            f3 = tpool.tile([HB, FD], F16, tag="n")
            v.tensor_tensor(f3[:], f1[:], h6[:], ALU.mult)
            axxx = spool.tile([HB, FD], F16, tag="axxx")
            v.tensor_scalar(axxx[:], f3[:], cx3, None, ALU.mult)

            # ---------- layers 2..5 ----------
            for layer in range(2, 6):
                W = wl(layer)
                last = layer == 5

                # PE: five jet matmuls (issue in stream-production order)
                z = zpool.tile([HB, FD], F32, tag="ztw")
                _mm_chunks(nc, z, W, a)
                zx = zpool.tile([HB, FD], F32, tag="ztw")
                _mm_chunks(nc, zx, W, ax)
                zt = zpool.tile([HB, FD], F32, tag="ztw")
                _mm_chunks(nc, zt, W, at)
                zxx = zpool.tile([HB, FD], F32, tag="ztw")
                _mm_chunks(nc, zxx, W, axx)
                zxxx = zpool.tile([HB, FD], F32, tag="ztw")
                _mm_chunks(nc, zxxx, W, axxx)

                # ACT: PSUM consumers + squares (GpSimd is intentionally
                # unused here: its SBUF traffic slows concurrent DVE ops ~3x)
                a_n = spool.tile([HB, FD], F16, tag="a")
                s.activation(a_n[:], z[:], AF.Tanh, bias=bb(layer))
                asq = tpool.tile([HB, FD], F16, tag="asq")
                s.activation(asq[:], a_n[:], AF.Square)
                dS = tpool.tile([HB, FD], F16, tag="dS")
                s.activation(dS[:], zx[:], AF.Copy)
                d2 = tpool.tile([HB, FD], F16, tag="d2")
                s.activation(d2[:], zx[:], AF.Square)
                cS = tpool.tile([HB, FD], F16, tag="cS")
                s.activation(cS[:], zxx[:], AF.Copy)

                # DVE fast fp16 chain
                f1 = tpool.tile([HB, FD], F16, tag="f1")
                v.tensor_scalar(f1[:], asq[:], -1.0, 1.0, ALU.mult, ALU.add)
                h6 = tpool.tile([HB, FD], F16, tag="h6")
                v.tensor_scalar(h6[:], asq[:], 6.0, -2.0, ALU.mult, ALU.add)
                ax_n = spool.tile([HB, FD], F16, tag="ax")
                v.tensor_tensor(ax_n[:], f1[:], dS[:], ALU.mult)
                at_n = spool.tile([HB, FD], F16, tag="at")
                v.tensor_tensor(at_n[:], f1[:], zt[:], ALU.mult)
                d3 = tpool.tile([HB, FD], F16, tag="d3")
                v.tensor_tensor(d3[:], d2[:], dS[:], ALU.mult)
                dc = tpool.tile([HB, FD], F16, tag="dc")
                v.tensor_tensor(dc[:], dS[:], cS[:], ALU.mult)
                if not last:
                    t2 = tpool.tile([HB, FD], F16, tag="t2")
                    v.tensor_tensor(t2[:], a_n[:], d2[:], ALU.mult)
                m = tpool.tile([HB, FD], F16, tag="m")
                v.tensor_tensor(m[:], a_n[:], dc[:], ALU.mult)
                n_t = tpool.tile([HB, FD], F16, tag="n")
                v.tensor_tensor(n_t[:], h6[:], d3[:], ALU.mult)
                if not last:
                    inner = tpool.tile([HB, FD], F16, tag="inner")
                    v.scalar_tensor_tensor(inner[:], t2[:], -2.0, cS[:],
                                           ALU.mult, ALU.add)
                i3a = tpool.tile([HB, FD], F16, tag="i3a")
                v.scalar_tensor_tensor(i3a[:], m[:], -6.0, zxxx[:],
                                       ALU.mult, ALU.add)
                if not last:
                    axx_n = spool.tile([HB, FD], F16, tag="axx")
                    v.tensor_tensor(axx_n[:], f1[:], inner[:], ALU.mult)
                i3 = tpool.tile([HB, FD], F16, tag="i3")
                v.tensor_tensor(i3[:], i3a[:], n_t[:], ALU.add)
                axxx_n = spool.tile([HB, FD], F16, tag="axxx")
                v.tensor_tensor(axxx_n[:], f1[:], i3[:], ALU.mult)

                a, at, ax, axxx = a_n, at_n, ax_n, axxx_n
                if not last:
                    axx = axx_n

            a5, ax5, at5, axxx5 = a, ax, at, axxx

        if stage == "tower":
            dbgS = cpool.tile([HB, FD], F32, tag="dbgS")
            v.tensor_copy(dbgS[:], axxx5[:])
            nc.sync.dma_start(loss_d[:], dbgS[:])
            return

        # ---------- layer 6 + Gram ----------
        # chunk tiles: [128 points, 10] cols: s-major pairs (b0,b1) for
        # s=0 uux, 1 uxxx, 2 ux, 3 ut; cols 8:10 = u.
        with tc.tile_pool(name="l6c", bufs=2, space="PSUM") as l6p, \
             tc.tile_pool(name="psmall", bufs=1, space="PSUM") as pps:
            G = pps.tile([4, 4], F32, tag="gram")
            for c in range(5):
                lo = 128 * c
                ch = l6p.tile([128, 10], F32, tag="l6c")
                nc.tensor.matmul(ch[:, 8:10], a5[:, lo:lo + 128], w6p)
                nc.tensor.matmul(ch[:, 2:4], axxx5[:, lo:lo + 128], w6p)
                nc.tensor.matmul(ch[:, 4:6], ax5[:, lo:lo + 128], w6p)
                nc.tensor.matmul(ch[:, 6:8], at5[:, lo:lo + 128], w6p)
                chS = tpool.tile([128, 10], F16, tag="l6s")
                v.tensor_copy(chS[:, 2:8], ch[:, 2:8])
                # uux = (u + b6) * ux
                v.scalar_tensor_tensor(chS[:, 0:2], ch[:, 8:10], b6,
                                       chS[:, 4:6], ALU.add, ALU.mult)
                chv = chS[:, 0:8].rearrange("p (s b) -> p b s", b=2, s=4)
                if c == 4 and B1 < FD:
                    # zero the padded block1 points before the Gram matmul
                    v.tensor_scalar(chv[:, 1, :], chv[:, 1, :], msk,
                                    None, ALU.mult)
                for b in range(2):
                    st = c == 0 and b == 0
                    sp = c == 4 and b == 1
                    nc.tensor.matmul(G[:], chv[:, b, :], chv[:, b, :],
                                     start=st, stop=sp)

            gS = cpool.tile([4, 4], F32, tag="gS")
            v.tensor_copy(gS[:], G[:])

            if stage == "l6":
                nc.sync.dma_start(loss_d[:], gS[:])
                return

            # ---------- AllReduce the Gram ----------
            gin = dpool.tile([4, 4], F32, tag="gin")
            gout = dpool.tile([4, 4], F32, tag="gout")
            nc.gpsimd.dma_start(gin[:], gS[:])
            nc.gpsimd.collective_compute(
                "AllReduce",
                ALU.add,
                replica_groups=[list(range(NCORES))],
                ins=[gin.opt()],
                outs=[gout.opt()],
            )
            # read back as [16,1]: one Gram value per partition
            gF = cpool.tile([16, 1], F32, tag="gF")
            nc.sync.dma_start(gF[:], gout[:])

            # ---------- loss = (gvec/N)^T @ Q16 ----------
            gv = cpool.tile([16, 1], F16, tag="gv")
            v.tensor_scalar(gv[:], gF[:], 1.0 / NPTS, None, ALU.mult)
            P = pps.tile([1, PPC], F32, tag="lossP")
            for off, w in QCH:
                nc.tensor.matmul(P[:, off:off + w], gv[:], q16[:, off:off + w])
            lossS = cpool.tile([1, PPC], F32, tag="lossS")
            v.tensor_copy(lossS[:], P[:])
            nc.sync.dma_start(loss_d[:], lossS[:])


def prep_inputs(x, para, W1, b1, W2, b2, W3, b3, W4, b4, W5, b5, W6, b6):
    """Full inputs -> list of per-core input dicts (host-side shard/layout)."""
    f = np.float32
    h = np.float16
    x = np.asarray(x, f)
    para = np.asarray(para, f)
    Ws = [np.asarray(W, f) for W in (W1, W2, W3, W4, W5, W6)]
    bs = [np.asarray(b, f) for b in (b1, b2, b3, b4, b5, b6)]

    # blob A: block-diagonal tower weights + layer-6 projection, fp16
    A = np.zeros((HB, 402), h)
    for i in range(4):
        W = Ws[i + 1]
        A[0:50, 100 * i:100 * i + 50] = W.T
        A[50:100, 100 * i + 50:100 * i + 100] = W.T
    A[0:50, 400] = Ws[5][0]
    A[50:100, 401] = Ws[5][0]

    # blob F: f32 per-partition scalars (layer-1 jet coefficients, biases,
    # layer-6 bias + block1 padding mask)
    Ff = np.zeros((128, 12), f)
    cx = Ws[0][:, 0]
    ct = Ws[0][:, 1]
    for half in (slice(0, 50), slice(50, 100)):
        Ff[half, 0] = cx
        Ff[half, 1] = ct
        Ff[half, 2] = cx * cx
        Ff[half, 3] = cx * cx * cx
        for l in range(5):
            Ff[half, 4 + l] = bs[l]
    Ff[:, 10] = bs[5][0]
    Ff[:, 11] = 1.0
    Ff[B1 - 512:, 11] = 0.0

    maps = []
    for c in range(NCORES):
        sl = x[c * NPC:(c + 1) * NPC]
        # blob B: per-core points + W1 + para features
        Bb = np.zeros((16, 1365), h)
        Bb[0, 0:FD] = sl[0:FD, 0]
        Bb[1, 0:FD] = sl[0:FD, 1]
        Bb[2, 0:B1] = sl[FD:NPC, 0]
        Bb[3, 0:B1] = sl[FD:NPC, 1]
        Bb[0:2, 640:690] = Ws[0].T
        Bb[2:4, 690:740] = Ws[0].T
        pc = para[c * PPC:(c + 1) * PPC]          # [625, 3]
        pp = np.concatenate([pc.T, np.ones((1, PPC), f)], axis=0)  # [4,625]
        for i in range(4):
            for j in range(4):
                Bb[4 * i + j, 740:1365] = pp[i] * pp[j]
        maps.append({"cstA": A, "cstB": Bb, "cstF": Ff})
    return maps


_NC_CACHE = {}


def get_program():
    if "nc" not in _NC_CACHE:
        _NC_CACHE["nc"] = build_program()
    return _NC_CACHE["nc"]


def kernel(x, para, W1, b1, W2, b2, W3, b3, W4, b4, W5, b5, W6, b6):
    maps = prep_inputs(x, para, W1, b1, W2, b2, W3, b3, W4, b4, W5, b5, W6, b6)
    nc = get_program()
    res = bass_utils.run_bass_kernel_spmd(nc, maps, list(range(NCORES)))
    out = np.concatenate([res.results[c]["loss"].reshape(-1) for c in range(NCORES)])
    return out.astype(np.float32)


# revision 9
# speedup vs baseline: 1.0974x; 1.0974x over previous
"""Trainium2 Bass kernel for the PINN-style loss problem (v2, fp16 tower).

Math: a 6-layer tanh MLP u(x,t) (2->50x5->1) is evaluated with forward-mode
jets (u, u_x, u_t, u_xxx) at N=10000 points. The per-param loss
  loss_p = mean_n (u_t + a_p*u*u_x + b_p*u_xxx + c_p*u_x)^2
collapses to a quadratic form in the 4x4 Gram matrix of
v_n = [u*u_x, u_xxx, u_x, u_t]:  loss_p = sum_ij p_i p_j G_ij / N with
p = [a_p, b_p, c_p, 1].

Sharding: x is split into 8 slices of 1250 points (one per NeuronCore);
each core builds its partial Gram, an AllReduce sums them, then each core
contracts the global Gram against host-precomputed para features
Q16[16,625] (rows p_i*p_j) with a single tiny matmul.

Device layout: points are packed 2-per-partition-block (block-diagonal
weights, K=100), free dim 640 per block (block0: 640 real points,
block1: 610 real + 30 zero-padded, masked out before the Gram matmul).
The jet pipeline runs in fp16 (DVE 2x / PE 16-bit rates); PSUM stays f32.
"""

import os
import sys
import numpy as np

for _p in ("/opt/trn_rl_repo",):
    if os.path.isdir(_p) and _p not in sys.path:
        sys.path.append(_p)

import concourse.bass as bass
import concourse.bacc as bacc
import concourse.mybir as mybir
import concourse.tile as tile
from concourse import bass_utils

F32 = mybir.dt.float32
F16 = mybir.dt.float16
AF = mybir.ActivationFunctionType
ALU = mybir.AluOpType

NCORES = 8
NPTS = 10000
NPC = NPTS // NCORES       # 1250 points per core
PPC = 5000 // NCORES       # 625 para rows per core
FD = 640                   # free dim per block (block0 full, block1 padded)
B1 = NPC - FD              # 610 real points in block1
HB = 100                   # 2 blocks x 50 hidden units
CHUNKS = ((0, 512), (512, 128))      # matmul free-dim chunks (psum bank limit)
QCH = ((0, 512), (512, PPC - 512))   # loss free-dim chunks

WARM_CC = True             # early dummy collective to warm the CC path


def _mm_chunks(nc, out_tile, lhsT, rhs_tile, chunks=CHUNKS):
    for off, w in chunks:
        nc.tensor.matmul(out_tile[:, off:off + w], lhsT, rhs_tile[:, off:off + w])


def build_program(stage="full"):
    nc = bacc.Bacc("TRN2", target_bir_lowering=False, debug=False)

    cstA_d = nc.dram_tensor("cstA", [HB, 402], F16, kind="ExternalInput")
    cstB_d = nc.dram_tensor("cstB", [16, 1365], F16, kind="ExternalInput")
    cstF_d = nc.dram_tensor("cstF", [128, 12], F32, kind="ExternalInput")
    if stage == "tower":
        loss_d = nc.dram_tensor("dbg", [HB, FD], F32, kind="ExternalOutput")
    elif stage == "l6":
        loss_d = nc.dram_tensor("dbg", [4, 4], F32, kind="ExternalOutput")
    else:
        loss_d = nc.dram_tensor("loss", [1, PPC], F32, kind="ExternalOutput")

    with tile.TileContext(nc) as tc:
        _body(tc, nc, cstA_d, cstB_d, cstF_d, loss_d, stage=stage)
    nc.compile()
    return nc


def _body(tc, nc, cstA_d, cstB_d, cstF_d, loss_d, stage="full"):
    import contextlib

    ctx = contextlib.ExitStack()
    with ctx:
        cpool = ctx.enter_context(tc.tile_pool(name="const", bufs=1))
        spool = ctx.enter_context(tc.tile_pool(name="streams", bufs=2))
        tpool = ctx.enter_context(tc.tile_pool(name="trans", bufs=2))
        dpool = ctx.enter_context(tc.tile_pool(name="dram", bufs=1, space="DRAM"))

        # ---- load constants (3 batched DMAs) ----
        A = cpool.tile([HB, 402], F16, tag="cstA")
        B = cpool.tile([16, 1365], F16, tag="cstB")
        F = cpool.tile([128, 12], F32, tag="cstF")
        nc.sync.dma_start(B[:], cstB_d[:])
        nc.sync.dma_start(A[:], cstA_d[:])
        nc.sync.dma_start(F[:], cstF_d[:])

        h0 = B[0:4, 0:640]
        w1t = B[0:4, 640:740]
        q16 = B[:, 740:1365]
        w6p = A[:, 400:402]

        def wl(layer):  # weight block for layer 2..5
            return A[:, 100 * (layer - 2):100 * (layer - 1)]

        cx = F[0:HB, 0:1]
        ct = F[0:HB, 1:2]
        cx2 = F[0:HB, 2:3]
        cx3 = F[0:HB, 3:4]

        def bb(layer):  # bias vector for layer 1..5
            return F[0:HB, 3 + layer:4 + layer]

        b6 = F[:, 10:11]
        msk = F[:, 11:12]

        wone = cpool.tile([1, 1], F32, tag="wone")
        nc.vector.memset(wone[:], 1.0)

        if WARM_CC:
            win = dpool.tile([1, 1], F32, tag="win")
            wout = dpool.tile([1, 1], F32, tag="wout")
            nc.gpsimd.dma_start(win[:], wone[:])
            nc.gpsimd.collective_compute(
                "AllReduce", ALU.add,
                replica_groups=[list(range(NCORES))],
                ins=[win.opt()], outs=[wout.opt()],
            )

        v = nc.vector
        s = nc.scalar
        g = nc.gpsimd

        a5 = ax5 = at5 = axxx5 = None

        with tc.tile_pool(name="ztw", bufs=3, space="PSUM") as zpool:
            # ---------- layer 1 ----------
            # zx/zt are constant per hidden unit: cx/ct. Jets come from
            # tensor_scalar ops with the per-partition weight columns.
            z = zpool.tile([HB, FD], F32, tag="ztw")
            _mm_chunks(nc, z, w1t, h0)
            a = spool.tile([HB, FD], F16, tag="a")
            s.activation(a[:], z[:], AF.Tanh, bias=bb(1))
            asq = tpool.tile([HB, FD], F16, tag="asq")
            s.activation(asq[:], a[:], AF.Square)
            f1 = tpool.tile([HB, FD], F16, tag="f1")
            v.tensor_scalar(f1[:], asq[:], -1.0, 1.0, ALU.mult, ALU.add)
            h6 = tpool.tile([HB, FD], F16, tag="h6")
            v.tensor_scalar(h6[:], asq[:], 6.0, -2.0, ALU.mult, ALU.add)
            ax = spool.tile([HB, FD], F16, tag="ax")
            v.tensor_scalar(ax[:], f1[:], cx, None, ALU.mult)
            at = spool.tile([HB, FD], F16, tag="at")
            v.tensor_scalar(at[:], f1[:], ct, None, ALU.mult)
            af1 = tpool.tile([HB, FD], F16, tag="t2")
            v.tensor_tensor(af1[:], a[:], f1[:], ALU.mult)
            axx = spool.tile([HB, FD], F16, tag="axx")
            v.tensor_scalar(axx[:], af1[:], cx2, -2.0, ALU.mult, ALU.mult)
            f3 = tpool.tile([HB, FD], F16, tag="n")
            v.tensor_tensor(f3[:], f1[:], h6[:], ALU.mult)
            axxx = spool.tile([HB, FD], F16, tag="axxx")
            v.tensor_scalar(axxx[:], f3[:], cx3, None, ALU.mult)

            # ---------- layers 2..5 ----------
            for layer in range(2, 6):
                W = wl(layer)
                last = layer == 5

                # PE: five jet matmuls (issue in stream-production order)
                z = zpool.tile([HB, FD], F32, tag="ztw")
                _mm_chunks(nc, z, W, a)
                zx = zpool.tile([HB, FD], F32, tag="ztw")
                _mm_chunks(nc, zx, W, ax)
                zt = zpool.tile([HB, FD], F32, tag="ztw")
                _mm_chunks(nc, zt, W, at)
                zxx = zpool.tile([HB, FD], F32, tag="ztw")
                _mm_chunks(nc, zxx, W, axx)
                zxxx = zpool.tile([HB, FD], F32, tag="ztw")
                _mm_chunks(nc, zxxx, W, axxx)

                # ACT: PSUM consumers + squares (GpSimd is intentionally
                # unused here: its SBUF traffic slows concurrent DVE ops ~3x)
                a_n = spool.tile([HB, FD], F16, tag="a")
                s.activation(a_n[:], z[:], AF.Tanh, bias=bb(layer))
                asq = tpool.tile([HB, FD], F16, tag="asq")
                s.activation(asq[:], a_n[:], AF.Square)
                dS = tpool.tile([HB, FD], F16, tag="dS")
                s.activation(dS[:], zx[:], AF.Copy)
                d2 = tpool.tile([HB, FD], F16, tag="d2")
                s.activation(d2[:], zx[:], AF.Square)
                cS = tpool.tile([HB, FD], F16, tag="cS")
                s.activation(cS[:], zxx[:], AF.Copy)
                sS = tpool.tile([HB, FD], F16, tag="sS")
                s.activation(sS[:], zt[:], AF.Copy)
                qS = tpool.tile([HB, FD], F16, tag="qS")
                s.activation(qS[:], zxxx[:], AF.Copy)

                # DVE fast fp16 chain
                f1 = tpool.tile([HB, FD], F16, tag="f1")
                v.tensor_scalar(f1[:], asq[:], -1.0, 1.0, ALU.mult, ALU.add)
                h6 = tpool.tile([HB, FD], F16, tag="h6")
                v.tensor_scalar(h6[:], asq[:], 6.0, -2.0, ALU.mult, ALU.add)
                ax_n = spool.tile([HB, FD], F16, tag="ax")
                v.tensor_tensor(ax_n[:], f1[:], dS[:], ALU.mult)
                d3 = tpool.tile([HB, FD], F16, tag="d3")
                v.tensor_tensor(d3[:], d2[:], dS[:], ALU.mult)
                dc = tpool.tile([HB, FD], F16, tag="dc")
                v.tensor_tensor(dc[:], dS[:], cS[:], ALU.mult)
                if not last:
                    t2 = tpool.tile([HB, FD], F16, tag="t2")
                    v.tensor_tensor(t2[:], a_n[:], d2[:], ALU.mult)
                m = tpool.tile([HB, FD], F16, tag="m")
                v.tensor_tensor(m[:], a_n[:], dc[:], ALU.mult)
                at_n = spool.tile([HB, FD], F16, tag="at")
                v.tensor_tensor(at_n[:], f1[:], sS[:], ALU.mult)
                n_t = tpool.tile([HB, FD], F16, tag="n")
                v.tensor_tensor(n_t[:], h6[:], d3[:], ALU.mult)
                if not last:
                    inner = tpool.tile([HB, FD], F16, tag="inner")
                    v.scalar_tensor_tensor(inner[:], t2[:], -2.0, cS[:],
                                           ALU.mult, ALU.add)
                i3a = tpool.tile([HB, FD], F16, tag="i3a")
                v.scalar_tensor_tensor(i3a[:], m[:], -6.0, qS[:],
                                       ALU.mult, ALU.add)
                if not last:
                    axx_n = spool.tile([HB, FD], F16, tag="axx")
                    v.tensor_tensor(axx_n[:], f1[:], inner[:], ALU.mult)
                i3 = tpool.tile([HB, FD], F16, tag="i3")
                v.tensor_tensor(i3[:], i3a[:], n_t[:], ALU.add)
                axxx_n = spool.tile([HB, FD], F16, tag="axxx")
                v.tensor_tensor(axxx_n[:], f1[:], i3[:], ALU.mult)

                a, at, ax, axxx = a_n, at_n, ax_n, axxx_n
                if not last:
                    axx = axx_n

            a5, ax5, at5, axxx5 = a, ax, at, axxx

        if stage == "tower":
            dbgS = cpool.tile([HB, FD], F32, tag="dbgS")
            v.tensor_copy(dbgS[:], axxx5[:])
            nc.sync.dma_start(loss_d[:], dbgS[:])
            return

        # ---------- layer 6 + Gram ----------
        # chunk tiles: [128 points, 10] cols: s-major pairs (b0,b1) for
        # s=0 uux, 1 uxxx, 2 ux, 3 ut; cols 8:10 = u.
        with tc.tile_pool(name="l6c", bufs=2, space="PSUM") as l6p, \
             tc.tile_pool(name="psmall", bufs=1, space="PSUM") as pps:
            G = pps.tile([4, 4], F32, tag="gram")
            for c in range(5):
                lo = 128 * c
                ch = l6p.tile([128, 10], F32, tag="l6c")
                nc.tensor.matmul(ch[:, 8:10], a5[:, lo:lo + 128], w6p)
                nc.tensor.matmul(ch[:, 2:4], axxx5[:, lo:lo + 128], w6p)
                nc.tensor.matmul(ch[:, 4:6], ax5[:, lo:lo + 128], w6p)
                nc.tensor.matmul(ch[:, 6:8], at5[:, lo:lo + 128], w6p)
                chS = tpool.tile([128, 10], F16, tag="l6s")
                v.tensor_copy(chS[:, 2:8], ch[:, 2:8])
                # uux = (u + b6) * ux
                v.scalar_tensor_tensor(chS[:, 0:2], ch[:, 8:10], b6,
                                       chS[:, 4:6], ALU.add, ALU.mult)
                chv = chS[:, 0:8].rearrange("p (s b) -> p b s", b=2, s=4)
                if c == 4 and B1 < FD:
                    # zero the padded block1 points before the Gram matmul
                    v.tensor_scalar(chv[:, 1, :], chv[:, 1, :], msk,
                                    None, ALU.mult)
                for b in range(2):
                    st = c == 0 and b == 0
                    sp = c == 4 and b == 1
                    nc.tensor.matmul(G[:], chv[:, b, :], chv[:, b, :],
                                     start=st, stop=sp)

            gS = cpool.tile([4, 4], F32, tag="gS")
            v.tensor_copy(gS[:], G[:])

            if stage == "l6":
                nc.sync.dma_start(loss_d[:], gS[:])
                return

            # ---------- AllReduce the Gram ----------
            gin = dpool.tile([4, 4], F32, tag="gin")
            gout = dpool.tile([4, 4], F32, tag="gout")
            nc.gpsimd.dma_start(gin[:], gS[:])
            nc.gpsimd.collective_compute(
                "AllReduce",
                ALU.add,
                replica_groups=[list(range(NCORES))],
                ins=[gin.opt()],
                outs=[gout.opt()],
            )
            # read back as [16,1]: one Gram value per partition
            gF = cpool.tile([16, 1], F32, tag="gF")
            nc.sync.dma_start(gF[:], gout[:])

            # ---------- loss = (gvec/N)^T @ Q16 ----------
            gv = cpool.tile([16, 1], F16, tag="gv")
            v.tensor_scalar(gv[:], gF[:], 1.0 / NPTS, None, ALU.mult)
            P = pps.tile([1, PPC], F32, tag="lossP")
            for off, w in QCH:
                nc.tensor.matmul(P[:, off:off + w], gv[:], q16[:, off:off + w])
            lossS = cpool.tile([1, PPC], F32, tag="lossS")
            v.tensor_copy(lossS[:], P[:])
            nc.sync.dma_start(loss_d[:], lossS[:])


def prep_inputs(x, para, W1, b1, W2, b2, W3, b3, W4, b4, W5, b5, W6, b6):
    """Full inputs -> list of per-core input dicts (host-side shard/layout)."""
    f = np.float32
    h = np.float16
    x = np.asarray(x, f)
    para = np.asarray(para, f)
    Ws = [np.asarray(W, f) for W in (W1, W2, W3, W4, W5, W6)]
    bs = [np.asarray(b, f) for b in (b1, b2, b3, b4, b5, b6)]

    # blob A: block-diagonal tower weights + layer-6 projection, fp16
    A = np.zeros((HB, 402), h)
    for i in range(4):
        W = Ws[i + 1]
        A[0:50, 100 * i:100 * i + 50] = W.T
        A[50:100, 100 * i + 50:100 * i + 100] = W.T
    A[0:50, 400] = Ws[5][0]
    A[50:100, 401] = Ws[5][0]

    # blob F: f32 per-partition scalars (layer-1 jet coefficients, biases,
    # layer-6 bias + block1 padding mask)
    Ff = np.zeros((128, 12), f)
    cx = Ws[0][:, 0]
    ct = Ws[0][:, 1]
    for half in (slice(0, 50), slice(50, 100)):
        Ff[half, 0] = cx
        Ff[half, 1] = ct
        Ff[half, 2] = cx * cx
        Ff[half, 3] = cx * cx * cx
        for l in range(5):
            Ff[half, 4 + l] = bs[l]
    Ff[:, 10] = bs[5][0]
    Ff[:, 11] = 1.0
    Ff[B1 - 512:, 11] = 0.0

    maps = []
    for c in range(NCORES):
        sl = x[c * NPC:(c + 1) * NPC]
        # blob B: per-core points + W1 + para features
        Bb = np.zeros((16, 1365), h)
        Bb[0, 0:FD] = sl[0:FD, 0]
        Bb[1, 0:FD] = sl[0:FD, 1]
        Bb[2, 0:B1] = sl[FD:NPC, 0]
        Bb[3, 0:B1] = sl[FD:NPC, 1]
        Bb[0:2, 640:690] = Ws[0].T
        Bb[2:4, 690:740] = Ws[0].T
        pc = para[c * PPC:(c + 1) * PPC]          # [625, 3]
        pp = np.concatenate([pc.T, np.ones((1, PPC), f)], axis=0)  # [4,625]
        for i in range(4):
            for j in range(4):
                Bb[4 * i + j, 740:1365] = pp[i] * pp[j]
        maps.append({"cstA": A, "cstB": Bb, "cstF": Ff})
    return maps


_NC_CACHE = {}


def get_program():
    if "nc" not in _NC_CACHE:
        _NC_CACHE["nc"] = build_program()
    return _NC_CACHE["nc"]


def kernel(x, para, W1, b1, W2, b2, W3, b3, W4, b4, W5, b5, W6, b6):
    maps = prep_inputs(x, para, W1, b1, W2, b2, W3, b3, W4, b4, W5, b5, W6, b6)
    nc = get_program()
    res = bass_utils.run_bass_kernel_spmd(nc, maps, list(range(NCORES)))
    out = np.concatenate([res.results[c]["loss"].reshape(-1) for c in range(NCORES)])
    return out.astype(np.float32)


# revision 10
# speedup vs baseline: 1.3290x; 1.2110x over previous
"""Trainium2 Bass kernel for the PINN-style loss problem (v2, fp16 tower).

Math: a 6-layer tanh MLP u(x,t) (2->50x5->1) is evaluated with forward-mode
jets (u, u_x, u_t, u_xxx) at N=10000 points. The per-param loss
  loss_p = mean_n (u_t + a_p*u*u_x + b_p*u_xxx + c_p*u_x)^2
collapses to a quadratic form in the 4x4 Gram matrix of
v_n = [u*u_x, u_xxx, u_x, u_t]:  loss_p = sum_ij p_i p_j G_ij / N with
p = [a_p, b_p, c_p, 1].

Sharding: x is split into 8 slices of 1250 points (one per NeuronCore);
each core builds its partial Gram, an AllReduce sums them, then each core
contracts the global Gram against host-precomputed para features
Q16[16,625] (rows p_i*p_j) with a single tiny matmul.

Device layout: points are packed 2-per-partition-block (block-diagonal
weights, K=100), free dim 640 per block (block0: 640 real points,
block1: 610 real + 30 zero-padded, masked out before the Gram matmul).
The jet pipeline runs in fp16 (DVE 2x / PE 16-bit rates); PSUM stays f32.
"""

import os
import sys
import numpy as np

for _p in ("/opt/trn_rl_repo",):
    if os.path.isdir(_p) and _p not in sys.path:
        sys.path.append(_p)

import concourse.bass as bass
import concourse.bacc as bacc
import concourse.mybir as mybir
import concourse.tile as tile
from concourse import bass_utils

F32 = mybir.dt.float32
F16 = mybir.dt.float16
AF = mybir.ActivationFunctionType
ALU = mybir.AluOpType

NCORES = 8
NPTS = 10000
NPC = NPTS // NCORES       # 1250 points per core
PPC = 5000 // NCORES       # 625 para rows per core
FD = 640                   # free dim per block (block0 full, block1 padded)
B1 = NPC - FD              # 610 real points in block1
HB = 100                   # 2 blocks x 50 hidden units
CHUNKS = ((0, 512), (512, 128))      # matmul free-dim chunks (psum bank limit)
QCH = ((0, 512), (512, PPC - 512))   # loss free-dim chunks

WARM_CC = False            # early dummy collective to warm the CC path


def _mm_chunks(nc, out_tile, lhsT, rhs_tile, chunks=CHUNKS):
    for off, w in chunks:
        nc.tensor.matmul(out_tile[:, off:off + w], lhsT, rhs_tile[:, off:off + w])


def build_program(stage="full"):
    nc = bacc.Bacc("TRN2", target_bir_lowering=False, debug=False)

    cstA_d = nc.dram_tensor("cstA", [HB, 402], F16, kind="ExternalInput")
    cstB_d = nc.dram_tensor("cstB", [16, 1365], F16, kind="ExternalInput")
    cstF_d = nc.dram_tensor("cstF", [128, 12], F32, kind="ExternalInput")
    if stage == "tower":
        loss_d = nc.dram_tensor("dbg", [HB, FD], F32, kind="ExternalOutput")
    elif stage == "l6":
        loss_d = nc.dram_tensor("dbg", [4, 4], F32, kind="ExternalOutput")
    else:
        loss_d = nc.dram_tensor("loss", [1, PPC], F32, kind="ExternalOutput")

    with tile.TileContext(nc) as tc:
        _body(tc, nc, cstA_d, cstB_d, cstF_d, loss_d, stage=stage)
    nc.compile()
    return nc


def _body(tc, nc, cstA_d, cstB_d, cstF_d, loss_d, stage="full"):
    import contextlib

    ctx = contextlib.ExitStack()
    with ctx:
        cpool = ctx.enter_context(tc.tile_pool(name="const", bufs=1))
        spool = ctx.enter_context(tc.tile_pool(name="streams", bufs=2))
        tpool = ctx.enter_context(tc.tile_pool(name="trans", bufs=2))
        dpool = ctx.enter_context(tc.tile_pool(name="dram", bufs=1, space="DRAM"))

        # ---- load constants (3 batched DMAs) ----
        A = cpool.tile([HB, 402], F16, tag="cstA")
        B = cpool.tile([16, 1365], F16, tag="cstB")
        F = cpool.tile([128, 12], F32, tag="cstF")
        nc.sync.dma_start(B[:], cstB_d[:])
        nc.sync.dma_start(A[:], cstA_d[:])
        nc.sync.dma_start(F[:], cstF_d[:])

        h0 = B[0:4, 0:640]
        w1t = B[0:4, 640:740]
        q16 = B[:, 740:1365]
        w6p = A[:, 400:402]

        def wl(layer):  # weight block for layer 2..5
            return A[:, 100 * (layer - 2):100 * (layer - 1)]

        cx = F[0:HB, 0:1]
        ct = F[0:HB, 1:2]
        cx2 = F[0:HB, 2:3]
        cx3 = F[0:HB, 3:4]

        def bb(layer):  # bias vector for layer 1..5
            return F[0:HB, 3 + layer:4 + layer]

        b6 = F[:, 10:11]
        msk = F[:, 11:12]

        wone = cpool.tile([1, 1], F32, tag="wone")
        nc.vector.memset(wone[:], 1.0)

        if WARM_CC:
            win = dpool.tile([1, 1], F32, tag="win")
            wout = dpool.tile([1, 1], F32, tag="wout")
            nc.gpsimd.dma_start(win[:], wone[:])
            nc.gpsimd.collective_compute(
                "AllReduce", ALU.add,
                replica_groups=[list(range(NCORES))],
                ins=[win.opt()], outs=[wout.opt()],
            )

        v = nc.vector
        s = nc.scalar
        g = nc.gpsimd

        a5 = ax5 = at5 = axxx5 = None

        with tc.tile_pool(name="ztw", bufs=3, space="PSUM") as zpool:
            # ---------- layer 1 ----------
            # zx/zt are constant per hidden unit: cx/ct. Jets come from
            # tensor_scalar ops with the per-partition weight columns.
            z = zpool.tile([HB, FD], F32, tag="ztw")
            _mm_chunks(nc, z, w1t, h0)
            a = spool.tile([HB, FD], F16, tag="a")
            s.activation(a[:], z[:], AF.Tanh, bias=bb(1))
            asq = tpool.tile([HB, FD], F16, tag="asq")
            s.activation(asq[:], a[:], AF.Square)
            f1 = tpool.tile([HB, FD], F16, tag="f1")
            v.tensor_scalar(f1[:], asq[:], -1.0, 1.0, ALU.mult, ALU.add)
            h6 = tpool.tile([HB, FD], F16, tag="h6")
            v.tensor_scalar(h6[:], asq[:], 6.0, -2.0, ALU.mult, ALU.add)
            ax = spool.tile([HB, FD], F16, tag="ax")
            v.tensor_scalar(ax[:], f1[:], cx, None, ALU.mult)
            at = spool.tile([HB, FD], F16, tag="at")
            v.tensor_scalar(at[:], f1[:], ct, None, ALU.mult)
            af1 = tpool.tile([HB, FD], F16, tag="t2")
            v.tensor_tensor(af1[:], a[:], f1[:], ALU.mult)
            axx = spool.tile([HB, FD], F16, tag="axx")
            v.tensor_scalar(axx[:], af1[:], cx2, -2.0, ALU.mult, ALU.mult)
            f3 = tpool.tile([HB, FD], F16, tag="n")
            v.tensor_tensor(f3[:], f1[:], h6[:], ALU.mult)
            axxx = spool.tile([HB, FD], F16, tag="axxx")
            v.tensor_scalar(axxx[:], f3[:], cx3, None, ALU.mult)

            # ---------- layers 2..5 ----------
            for layer in range(2, 6):
                W = wl(layer)
                last = layer == 5

                # PE: five jet matmuls (issue in stream-production order)
                z = zpool.tile([HB, FD], F32, tag="ztw")
                _mm_chunks(nc, z, W, a)
                zx = zpool.tile([HB, FD], F32, tag="ztw")
                _mm_chunks(nc, zx, W, ax)
                zt = zpool.tile([HB, FD], F32, tag="ztw")
                _mm_chunks(nc, zt, W, at)
                zxx = zpool.tile([HB, FD], F32, tag="ztw")
                _mm_chunks(nc, zxx, W, axx)
                zxxx = zpool.tile([HB, FD], F32, tag="ztw")
                _mm_chunks(nc, zxxx, W, axxx)

                # ACT: PSUM consumers + squares (GpSimd is intentionally
                # unused here: its SBUF traffic slows concurrent DVE ops ~3x)
                a_n = spool.tile([HB, FD], F16, tag="a")
                s.activation(a_n[:], z[:], AF.Tanh, bias=bb(layer))
                asq = tpool.tile([HB, FD], F16, tag="asq")
                s.activation(asq[:], a_n[:], AF.Square)
                dS = tpool.tile([HB, FD], F16, tag="dS")
                s.activation(dS[:], zx[:], AF.Copy)
                d2 = tpool.tile([HB, FD], F16, tag="d2")
                s.activation(d2[:], zx[:], AF.Square)
                cS = tpool.tile([HB, FD], F16, tag="cS")
                s.activation(cS[:], zxx[:], AF.Copy)
                sS = tpool.tile([HB, FD], F16, tag="sS")
                s.activation(sS[:], zt[:], AF.Copy)
                qS = tpool.tile([HB, FD], F16, tag="qS")
                s.activation(qS[:], zxxx[:], AF.Copy)

                # DVE fast fp16 chain
                f1 = tpool.tile([HB, FD], F16, tag="f1")
                v.tensor_scalar(f1[:], asq[:], -1.0, 1.0, ALU.mult, ALU.add)
                h6 = tpool.tile([HB, FD], F16, tag="h6")
                v.tensor_scalar(h6[:], asq[:], 6.0, -2.0, ALU.mult, ALU.add)
                ax_n = spool.tile([HB, FD], F16, tag="ax")
                v.tensor_tensor(ax_n[:], f1[:], dS[:], ALU.mult)
                d3 = tpool.tile([HB, FD], F16, tag="d3")
                v.tensor_tensor(d3[:], d2[:], dS[:], ALU.mult)
                dc = tpool.tile([HB, FD], F16, tag="dc")
                v.tensor_tensor(dc[:], dS[:], cS[:], ALU.mult)
                if not last:
                    t2 = tpool.tile([HB, FD], F16, tag="t2")
                    v.tensor_tensor(t2[:], a_n[:], d2[:], ALU.mult)
                m = tpool.tile([HB, FD], F16, tag="m")
                v.tensor_tensor(m[:], a_n[:], dc[:], ALU.mult)
                at_n = spool.tile([HB, FD], F16, tag="at")
                v.tensor_tensor(at_n[:], f1[:], sS[:], ALU.mult)
                n_t = tpool.tile([HB, FD], F16, tag="n")
                v.tensor_tensor(n_t[:], h6[:], d3[:], ALU.mult)
                if not last:
                    inner = tpool.tile([HB, FD], F16, tag="inner")
                    v.scalar_tensor_tensor(inner[:], t2[:], -2.0, cS[:],
                                           ALU.mult, ALU.add)
                i3a = tpool.tile([HB, FD], F16, tag="i3a")
                v.scalar_tensor_tensor(i3a[:], m[:], -6.0, qS[:],
                                       ALU.mult, ALU.add)
                if not last:
                    axx_n = spool.tile([HB, FD], F16, tag="axx")
                    v.tensor_tensor(axx_n[:], f1[:], inner[:], ALU.mult)
                i3 = tpool.tile([HB, FD], F16, tag="i3")
                v.tensor_tensor(i3[:], i3a[:], n_t[:], ALU.add)
                axxx_n = spool.tile([HB, FD], F16, tag="axxx")
                v.tensor_tensor(axxx_n[:], f1[:], i3[:], ALU.mult)

                a, at, ax, axxx = a_n, at_n, ax_n, axxx_n
                if not last:
                    axx = axx_n

            a5, ax5, at5, axxx5 = a, ax, at, axxx

        if stage == "tower":
            dbgS = cpool.tile([HB, FD], F32, tag="dbgS")
            v.tensor_copy(dbgS[:], axxx5[:])
            nc.sync.dma_start(loss_d[:], dbgS[:])
            return

        # ---------- layer 6 + Gram ----------
        # chunk tiles: [128 points, 10] cols: s-major pairs (b0,b1) for
        # s=0 uux, 1 uxxx, 2 ux, 3 ut; cols 8:10 = u.
        with tc.tile_pool(name="l6c", bufs=2, space="PSUM") as l6p, \
             tc.tile_pool(name="psmall", bufs=1, space="PSUM") as pps:
            G = pps.tile([4, 4], F32, tag="gram")
            for c in range(5):
                lo = 128 * c
                ch = l6p.tile([128, 10], F32, tag="l6c")
                nc.tensor.matmul(ch[:, 8:10], a5[:, lo:lo + 128], w6p)
                nc.tensor.matmul(ch[:, 2:4], axxx5[:, lo:lo + 128], w6p)
                nc.tensor.matmul(ch[:, 4:6], ax5[:, lo:lo + 128], w6p)
                nc.tensor.matmul(ch[:, 6:8], at5[:, lo:lo + 128], w6p)
                chS = tpool.tile([128, 10], F16, tag="l6s")
                v.tensor_copy(chS[:, 2:8], ch[:, 2:8])
                # uux = (u + b6) * ux
                v.scalar_tensor_tensor(chS[:, 0:2], ch[:, 8:10], b6,
                                       chS[:, 4:6], ALU.add, ALU.mult)
                chv = chS[:, 0:8].rearrange("p (s b) -> p b s", b=2, s=4)
                if c == 4 and B1 < FD:
                    # zero the padded block1 points before the Gram matmul
                    v.tensor_scalar(chv[:, 1, :], chv[:, 1, :], msk,
                                    None, ALU.mult)
                for b in range(2):
                    st = c == 0 and b == 0
                    sp = c == 4 and b == 1
                    nc.tensor.matmul(G[:], chv[:, b, :], chv[:, b, :],
                                     start=st, stop=sp)

            gS = cpool.tile([4, 4], F32, tag="gS")
            v.tensor_copy(gS[:], G[:])

            if stage == "l6":
                nc.sync.dma_start(loss_d[:], gS[:])
                return

            # ---------- AllReduce the Gram ----------
            gin = dpool.tile([4, 4], F32, tag="gin")
            gout = dpool.tile([4, 4], F32, tag="gout")
            nc.gpsimd.dma_start(gin[:], gS[:])
            nc.gpsimd.collective_compute(
                "AllReduce",
                ALU.add,
                replica_groups=[list(range(NCORES))],
                ins=[gin.opt()],
                outs=[gout.opt()],
            )
            # read back as [16,1]: one Gram value per partition
            gF = cpool.tile([16, 1], F32, tag="gF")
            nc.sync.dma_start(gF[:], gout[:])

            # ---------- loss = (gvec/N)^T @ Q16 ----------
            gv = cpool.tile([16, 1], F16, tag="gv")
            v.tensor_scalar(gv[:], gF[:], 1.0 / NPTS, None, ALU.mult)
            P = pps.tile([1, PPC], F32, tag="lossP")
            for off, w in QCH:
                nc.tensor.matmul(P[:, off:off + w], gv[:], q16[:, off:off + w])
            lossS = cpool.tile([1, PPC], F32, tag="lossS")
            v.tensor_copy(lossS[:], P[:])
            nc.sync.dma_start(loss_d[:], lossS[:])


def prep_inputs(x, para, W1, b1, W2, b2, W3, b3, W4, b4, W5, b5, W6, b6):
    """Full inputs -> list of per-core input dicts (host-side shard/layout)."""
    f = np.float32
    h = np.float16
    x = np.asarray(x, f)
    para = np.asarray(para, f)
    Ws = [np.asarray(W, f) for W in (W1, W2, W3, W4, W5, W6)]
    bs = [np.asarray(b, f) for b in (b1, b2, b3, b4, b5, b6)]

    # blob A: block-diagonal tower weights + layer-6 projection, fp16
    A = np.zeros((HB, 402), h)
    for i in range(4):
        W = Ws[i + 1]
        A[0:50, 100 * i:100 * i + 50] = W.T
        A[50:100, 100 * i + 50:100 * i + 100] = W.T
    A[0:50, 400] = Ws[5][0]
    A[50:100, 401] = Ws[5][0]

    # blob F: f32 per-partition scalars (layer-1 jet coefficients, biases,
    # layer-6 bias + block1 padding mask)
    Ff = np.zeros((128, 12), f)
    cx = Ws[0][:, 0]
    ct = Ws[0][:, 1]
    for half in (slice(0, 50), slice(50, 100)):
        Ff[half, 0] = cx
        Ff[half, 1] = ct
        Ff[half, 2] = cx * cx
        Ff[half, 3] = cx * cx * cx
        for l in range(5):
            Ff[half, 4 + l] = bs[l]
    Ff[:, 10] = bs[5][0]
    Ff[:, 11] = 1.0
    Ff[B1 - 512:, 11] = 0.0

    maps = []
    for c in range(NCORES):
        sl = x[c * NPC:(c + 1) * NPC]
        # blob B: per-core points + W1 + para features
        Bb = np.zeros((16, 1365), h)
        Bb[0, 0:FD] = sl[0:FD, 0]
        Bb[1, 0:FD] = sl[0:FD, 1]
        Bb[2, 0:B1] = sl[FD:NPC, 0]
        Bb[3, 0:B1] = sl[FD:NPC, 1]
        Bb[0:2, 640:690] = Ws[0].T
        Bb[2:4, 690:740] = Ws[0].T
        pc = para[c * PPC:(c + 1) * PPC]          # [625, 3]
        pp = np.concatenate([pc.T, np.ones((1, PPC), f)], axis=0)  # [4,625]
        for i in range(4):
            for j in range(4):
                Bb[4 * i + j, 740:1365] = pp[i] * pp[j]
        maps.append({"cstA": A, "cstB": Bb, "cstF": Ff})
    return maps


_NC_CACHE = {}


def get_program():
    if "nc" not in _NC_CACHE:
        _NC_CACHE["nc"] = build_program()
    return _NC_CACHE["nc"]


def kernel(x, para, W1, b1, W2, b2, W3, b3, W4, b4, W5, b5, W6, b6):
    maps = prep_inputs(x, para, W1, b1, W2, b2, W3, b3, W4, b4, W5, b5, W6, b6)
    nc = get_program()
    res = bass_utils.run_bass_kernel_spmd(nc, maps, list(range(NCORES)))
    out = np.concatenate([res.results[c]["loss"].reshape(-1) for c in range(NCORES)])
    return out.astype(np.float32)
